# revision 13
# baseline (speedup 1.0000x reference)
"""Bass/Trainium2 kernel for BertLikeSelfAttention (tanh softcap + ReLU-softmax).

Sharding: tensor-parallel across heads. 16 heads / 8 cores = 2 heads per core.
Each core computes its 128 output channels; host concatenates.

Per-core layout choices (all chosen so that NO on-chip transposes are needed):
  - X is pre-transposed on host: xt[b] = X[b].T  -> [HID, S].
  - Q/K projections produce Q.T/K.T layout [o=128, s] directly.
  - V is produced in natural layout [s, o] (lhsT = X.T tiles), augmented with a
    ones column per head -> V_aug [s, 65]: the context matmul
    ctxT[d_aug, q] = V_aug.T @ T then carries row 64 = sum_k T[k, q], i.e. the
    ReLU-softmax denominators come for free.
  - Scores are computed transposed: T[k, q] = K @ Q.T (contract d=64).
  - tanh soft-capping + attention-mask add fused into a single ScalarE
    activation: tanh(raw/240 + mask/30); relu on VectorE.
  - Normalization: eps+reciprocal of the sums row (partition 64), DMA hop to
    partition 0, GPSIMD partition-broadcast, one DVE multiply per head.
  - All large matmuls run as float32r (full-rate; classic fp32 is 1/4 rate on
    the trn2 PE). Measured absmax error vs fp32 reference: ~3e-4 of scale.
"""

import math
from contextlib import ExitStack

import numpy as np

import concourse.bacc as bacc
import concourse.mybir as mybir
import concourse.tile as tile
from concourse.bass_utils import run_bass_kernel_spmd

B, S, HID = 4, 2048, 1024
NH, HD = 16, 64
NCORES = 8
CPC = HID // NCORES  # output channels per core = 128
LOGITS_CAP = 30.0
EPS = 1e-6
SCALE = 1.0 / (math.sqrt(HD) * LOGITS_CAP)  # applied to raw q.k scores
EPS_ADJ = EPS / LOGITS_CAP

F32 = mybir.dt.float32
F32R = mybir.dt.float32r

# All large matmuls use float32r (full-rate fp32 PE mode; classic fp32 runs
# at 1/4 rate on trn2). Measured end-to-end absmax error vs the fp32
# reference: ~3e-4 of output scale.

NKT = S // 128  # 16 key tiles
NQG = S // 512  # 4 query groups
NHT = HID // 128  # 8 hidden (contraction) tiles


def build_program(reps=1):
    import contextlib
    nc = bacc.Bacc("TRN2", target_bir_lowering=False, debug=False)

    xt_d = nc.dram_tensor("xt", [B, HID, S], F32R, kind="ExternalInput")
    wqt_d = nc.dram_tensor("wqt", [HID, CPC], F32R, kind="ExternalInput")
    wkt_d = nc.dram_tensor("wkt", [HID, CPC], F32R, kind="ExternalInput")
    wvt_d = nc.dram_tensor("wvt", [HID, CPC], F32R, kind="ExternalInput")
    bq_d = nc.dram_tensor("bqv", [CPC, 1], F32, kind="ExternalInput")
    bk_d = nc.dram_tensor("bkv", [CPC, 1], F32, kind="ExternalInput")
    bvb_d = nc.dram_tensor("bvb", [128, CPC], F32, kind="ExternalInput")
    mask_d = nc.dram_tensor("maskd", [B, S], F32, kind="ExternalInput")
    out_d = nc.dram_tensor("out_t", [B, 2, HD, S], F32, kind="ExternalOutput")

    TANH = mybir.ActivationFunctionType.Tanh

    with tile.TileContext(nc) as tc, ExitStack() as ctx:
        consts = ctx.enter_context(tc.tile_pool(name="consts", bufs=1))
        xt_pool = ctx.enter_context(tc.tile_pool(name="xtp", bufs=8))
        qk_pool = ctx.enter_context(tc.tile_pool(name="qkp", bufs=2))
        v_pool = ctx.enter_context(tc.tile_pool(name="vp", bufs=17))
        tt_pool = ctx.enter_context(tc.tile_pool(name="ttp", bufs=4))
        sm_pool = ctx.enter_context(tc.tile_pool(name="smp", bufs=2))
        ob_pool = ctx.enter_context(tc.tile_pool(name="obp", bufs=4))
        pproj = ctx.enter_context(tc.tile_pool(name="pproj", bufs=2, space="PSUM"))
        psc = ctx.enter_context(tc.tile_pool(name="psc", bufs=2, space="PSUM"))
        pctx = ctx.enter_context(tc.tile_pool(name="pctx", bufs=2, space="PSUM"))

        # --- constants ---
        wq_sb = consts.tile([128, NHT, 128], F32R, name="wq_sb")
        wk_sb = consts.tile([128, NHT, 128], F32R, name="wk_sb")
        wv_sb = consts.tile([128, NHT, 128], F32R, name="wv_sb")
        nc.sync.dma_start(wq_sb, wqt_d.rearrange("(j p) o -> p j o", p=128))
        nc.sync.dma_start(wk_sb, wkt_d.rearrange("(j p) o -> p j o", p=128))
        nc.sync.dma_start(wv_sb, wvt_d.rearrange("(j p) o -> p j o", p=128))
        bq_sb = consts.tile([CPC, 1], F32, name="bq_sb")
        bk_sb = consts.tile([CPC, 1], F32, name="bk_sb")
        bvb_sb = consts.tile([128, CPC], F32, name="bvb_sb")
        nc.sync.dma_start(bq_sb, bq_d[:, :])
        nc.sync.dma_start(bk_sb, bk_d[:, :])
        nc.sync.dma_start(bvb_sb, bvb_d[:, :])
        mask_sb = consts.tile([128, B, NKT], F32, name="mask_sb")
        nc.sync.dma_start(mask_sb, mask_d.rearrange("b (k p) -> p b k", p=128))

        loop_cm = tc.For_i(0, reps, 1) if reps > 1 else contextlib.nullcontext()
        with loop_cm:
          for b in range(B):
            # --- load X.T tiles for this batch ---
            xts = []
            for j in range(NHT):
                xtile = xt_pool.tile([128, S], F32R, name=f"xt_{b}_{j}", tag="xt")
                nc.sync.dma_start(xtile, xt_d[b, j * 128 : (j + 1) * 128, :])
                xts.append(xtile)

            # --- Q.T / K.T projections: out [o=128, s] ---
            qt = qk_pool.tile([128, S], F32R, name=f"qt_{b}", tag="qt")
            kt = qk_pool.tile([128, S], F32R, name=f"kt_{b}", tag="kt")
            for dst, w_sb, b_sb in ((qt, wq_sb, bq_sb), (kt, wk_sb, bk_sb)):
                for sg in range(NQG):
                    ps = pproj.tile([128, 512], F32, name=f"psq_{b}_{sg}", tag="proj")
                    for j in range(NHT):
                        nc.tensor.matmul(
                            ps,
                            w_sb[:, j, :],
                            xts[j][:, sg * 512 : (sg + 1) * 512],
                            start=(j == 0),
                            stop=(j == NHT - 1),
                        )
                    nc.vector.tensor_scalar_add(
                        dst[:, sg * 512 : (sg + 1) * 512], ps, b_sb
                    )

            # --- V projection, natural layout [s, o], with ones columns ---
            vs = []
            for st in range(NKT):
                ps = pproj.tile([128, 128], F32, name=f"psv_{b}_{st}", tag="proj")
                for j in range(NHT):
                    nc.tensor.matmul(
                        ps,
                        xts[j][:, st * 128 : (st + 1) * 128],
                        wv_sb[:, j, :],
                        start=(j == 0),
                        stop=(j == NHT - 1),
                    )
                v = v_pool.tile([128, 130], F32R, name=f"v_{b}_{st}", tag="v")
                nc.vector.tensor_add(v[:, 0:64], ps[:, 0:64], bvb_sb[:, 0:64])
                nc.vector.tensor_add(v[:, 65:129], ps[:, 64:128], bvb_sb[:, 64:128])
                # ones columns: memset can't write f32r; tensor_scalar can
                # (out = in*0 + 1)
                nc.vector.tensor_scalar(
                    v[:, 64:65], bvb_sb[:, 0:1], 0.0, 1.0,
                    mybir.AluOpType.mult, mybir.AluOpType.add,
                )
                nc.vector.tensor_scalar(
                    v[:, 129:130], bvb_sb[:, 0:1], 0.0, 1.0,
                    mybir.AluOpType.mult, mybir.AluOpType.add,
                )
                vs.append(v)

            # --- attention ---
            for qg in range(NQG):
                q0 = qg * 512
                cA = pctx.tile([65, 512], F32, name=f"cA_{b}_{qg}", tag="ctx")
                cB = pctx.tile([65, 512], F32, name=f"cB_{b}_{qg}", tag="ctx")
                for kb in range(NKT):
                    k0 = kb * 128
                    sps = psc.tile([128, 1024], F32, name=f"sps_{b}_{qg}_{kb}", tag="sc")
                    # transposed scores T[k, q] per head
                    nc.tensor.matmul(
                        sps[:, 0:512],
                        kt[0:64, k0 : k0 + 128],
                        qt[0:64, q0 : q0 + 512],
                        start=True,
                        stop=True,
                    )
                    nc.tensor.matmul(
                        sps[:, 512:1024],
                        kt[64:128, k0 : k0 + 128],
                        qt[64:128, q0 : q0 + 512],
                        start=True,
                        stop=True,
                    )
                    ttile = tt_pool.tile([128, 1024], F32R, name=f"tt_{b}_{qg}_{kb}", tag="tt")
                    nc.scalar.activation(
                        ttile, sps, TANH, bias=mask_sb[:, b, kb : kb + 1], scale=SCALE
                    )
                    nc.vector.tensor_scalar_max(ttile, ttile, 0.0)
                    nc.tensor.matmul(
                        cA,
                        vs[kb][:, 0:65],
                        ttile[:, 0:512],
                        start=(kb == 0),
                        stop=(kb == NKT - 1),
                    )
                    nc.tensor.matmul(
                        cB,
                        vs[kb][:, 65:130],
                        ttile[:, 512:1024],
                        start=(kb == 0),
                        stop=(kb == NKT - 1),
                    )

                # --- normalize + write out ---
                # evict+eps+recip the sums rows at partition 64, then DMA-hop
                # them to partition 0 (gpsimd partition_broadcast only reads
                # partition 0 of its input).
                sums = sm_pool.tile([65, 1024], F32, name=f"sums_{b}_{qg}", tag="sums")
                nc.vector.tensor_scalar_add(sums[64:65, 0:512], cA[64:65, :], EPS_ADJ)
                nc.vector.tensor_scalar_add(sums[64:65, 512:1024], cB[64:65, :], EPS_ADJ)
                nc.vector.reciprocal(sums[64:65, :], sums[64:65, :])
                hopA = sm_pool.tile([1, 512], F32, name=f"hopA_{b}_{qg}", tag="hopA")
                hopB = sm_pool.tile([1, 512], F32, name=f"hopB_{b}_{qg}", tag="hopB")
                nc.sync.dma_start(hopA, sums[64:65, 0:512])
                nc.sync.dma_start(hopB, sums[64:65, 512:1024])
                rbA = sm_pool.tile([64, 512], F32, name=f"rbA_{b}_{qg}", tag="rbA")
                rbB = sm_pool.tile([64, 512], F32, name=f"rbB_{b}_{qg}", tag="rbB")
                nc.gpsimd.partition_broadcast(rbA, hopA, channels=64)
                nc.gpsimd.partition_broadcast(rbB, hopB, channels=64)
                obA = ob_pool.tile([64, 512], F32, name=f"obA_{b}_{qg}", tag="obA")
                obB = ob_pool.tile([64, 512], F32, name=f"obB_{b}_{qg}", tag="obB")
                nc.vector.tensor_mul(obA, cA[0:64, :], rbA)
                nc.vector.tensor_mul(obB, cB[0:64, :], rbB)
                nc.sync.dma_start(out_d[b, 0, :, q0 : q0 + 512], obA)
                nc.sync.dma_start(out_d[b, 1, :, q0 : q0 + 512], obB)

    nc.compile()
    return nc


_CACHE = {}


def _get_nc():
    if "nc" not in _CACHE:
        _CACHE["nc"] = build_program()
    return _CACHE["nc"]


def kernel(hidden_states, attention_mask, Wq, bq, Wk, bk, Wv, bv):
    hidden_states = np.asarray(hidden_states, dtype=np.float32)
    attention_mask = np.asarray(attention_mask, dtype=np.float32)
    Wq = np.asarray(Wq, dtype=np.float32)
    Wk = np.asarray(Wk, dtype=np.float32)
    Wv = np.asarray(Wv, dtype=np.float32)
    bq = np.asarray(bq, dtype=np.float32)
    bk = np.asarray(bk, dtype=np.float32)
    bv = np.asarray(bv, dtype=np.float32)

    nc = _get_nc()

    xt = np.ascontiguousarray(hidden_states.transpose(0, 2, 1))  # [B, HID, S]
    maskd = np.ascontiguousarray(
        attention_mask.reshape(B, S) / np.float32(LOGITS_CAP)
    )

    in_maps = []
    for i in range(NCORES):
        lo, hi = i * CPC, (i + 1) * CPC
        in_maps.append(
            {
                "xt": xt,
                "wqt": np.ascontiguousarray(Wq[lo:hi, :].T),
                "wkt": np.ascontiguousarray(Wk[lo:hi, :].T),
                "wvt": np.ascontiguousarray(Wv[lo:hi, :].T),
                "bqv": np.ascontiguousarray(bq[lo:hi].reshape(CPC, 1)),
                "bkv": np.ascontiguousarray(bk[lo:hi].reshape(CPC, 1)),
                "bvb": np.ascontiguousarray(
                    np.tile(bv[lo:hi][None, :], (128, 1))
                ),
                "maskd": maskd,
            }
        )

    res = None
    last_err = None
    for attempt in range(3):
        try:
            res = run_bass_kernel_spmd(nc, in_maps, list(range(NCORES)))
            break
        except Exception as e:  # transient NRT/axon device errors: retry
            last_err = e
            import time as _time

            _time.sleep(2.0 * (attempt + 1))
    if res is None:
        raise last_err

    out = np.empty((B, S, HID), dtype=np.float32)
    for i in range(NCORES):
        o = res.results[i]["out_t"]  # [B, 2, HD, S]
        out[:, :, i * CPC : (i + 1) * CPC] = (
            o.transpose(0, 3, 1, 2).reshape(B, S, CPC)
        )
    return out
